# revision 41
# baseline (speedup 1.0000x reference)
"""Trainium2 Bass kernel for ragged subword mean pooling (nn_Bert).

Problem: out[b, j] = mean(bert_embedding[b, st_j:ed_j]) if (mask & ed>st) else 0
Shapes: bert_embedding [32, 1024, 768] f32, x_bert_offset [32, 768, 2] i32,
        x_mask [32, 768] i32 -> out [32, 768, 768] f32.

Strategy (pure data parallel, 4 batch rows per core on 8 cores):
  Spans are contiguous sorted segments, so per row the pooling is
  out = A.T @ E where A[s, j] = scale_j iff st_j <= s < ed_j
  (scale_j = valid/len folds the mean and mask directly into A).
  Each position s belongs to at most ONE word, so every A tile has at
  most one nonzero per partition row. The host ships just that
  (column, value) pair per position and the device reconstructs each
  [128, win] A window in a single fused DVE op against a constant
  column-index tile J:
      A[p, j] = (J[p, j] == idx_p) * val_p

  The kernel is memory-bound, so bytes are minimized three ways:
  1. 16-bit I/O: E is shipped as fp16 (host casts; ~5e-4 rel rounding)
     and the output is written back as fp16 and upcast on the host.
  2. Output compaction: ~37% of words are invalid (masked or empty
     span) and produce zero rows. The word axis is compacted to valid
     words only before building A; the device writes only the rows up
     to the per-slot max valid count (partial last tile) and the host
     scatters rows back to the full [W] axis (invalid rows are zeros).
  3. Position compaction: ~10% of subword positions belong to masked/
     empty words and never contribute. The host gathers only covered-
     by-valid positions to the front of each E row; the device reads
     the per-slot max covered count of lines (partial last k-tile).

  DMA efficiency: the host ships E pre-packed in SBUF layout
  (partition-major, position k*128+p at [r, p, k*D:(k+1)*D]) and the
  output is written partition-major and un-transposed on the host, so
  every DMA moves multi-KB contiguous lines per partition instead of
  1.5 KB strided ones (measured ~20% faster than the strided layout).

  The contraction runs on the PE in fp16 (full rate, f32 PSUM
  accumulate). PSUM is drained by alternating scalar/vector copies.
  Only (m, k) tile pairs whose word/position ranges intersect are
  computed; the active-pair hull is derived on the host from the
  actual offsets (a superset is always correct since A is 0 outside).
"""

import sys

if "/opt/trn_rl_repo" not in sys.path:
    sys.path.insert(0, "/opt/trn_rl_repo")

import numpy as np

B, S, W, D = 32, 1024, 768, 768
NCORES = 8
RPC = B // NCORES  # rows per core
KT = S // 128  # 8 k-tiles (positions)

_CACHE = {}


def build_program(pairs, repeat=1, drain="alt", io="ext", stage=3, nodma=False,
                  ebufs=4, abufs=8, psbufs=3, obufs=3, avbufs=2,
                  wide_out=False, one_e=False, wq=False, packed=True,
                  jf16=False, aeng="vector"):
    """Build the SPMD Bass program (one program, run on all 8 cores)."""
    import concourse.tile as tile
    from concourse import bacc, mybir

    kl, mw, mtiles = pairs[:3]
    maxnv = pairs[3] if len(pairs) > 3 else tuple(m * 128 for m in mtiles)
    # per-slot count of (position-compacted) E lines to read; <= S
    scs = pairs[4] if len(pairs) > 4 else (S,) * RPC
    MTC = max(mtiles)
    f16 = mybir.dt.float16
    f32 = mybir.dt.float32
    i32 = mybir.dt.int32
    AF = mybir.ActivationFunctionType
    OP = mybir.AluOpType

    nc = bacc.Bacc(
        "TRN2", target_bir_lowering=False, debug=False, num_devices=NCORES
    )

    if packed:
        # host pre-packs E into SBUF layout: partition-major, position
        # k*128+p at [r, p, k*D:(k+1)*D]; DMA lines are multi-KB contiguous
        E_in = nc.dram_tensor(
            "E_in", [RPC, 128, KT * D], f16, kind="ExternalInput"
        ).ap()
    else:
        E_in = nc.dram_tensor("E_in", [RPC, S, D], f16, kind="ExternalInput").ap()
    # packed per (r, k): column 2*(r*KT+k) = one-hot column index within the
    # A window (or -1), column +1 = A value (scale of the word at that
    # position, 0 if masked/empty/uncovered)
    av_in = nc.dram_tensor("av_in", [128, RPC * KT * 2], f32, kind="ExternalInput").ap()
    oshape = [RPC, 128, MTC * D] if packed else [RPC, MTC * 128, D]
    if io == "ext":
        out = nc.dram_tensor("out", oshape, f16, kind="ExternalOutput").ap()
        tok = None
    else:
        out = nc.dram_tensor("out_scratch", oshape, f16).ap()
        tok = nc.dram_tensor("tok", [128, 16], f32, kind="ExternalOutput").ap()
    outdma = not nodma
    wdma = (lambda o, i: nc.scalar.dma_start(o, i)) if wq else (
        lambda o, i: nc.sync.dma_start(o, i)
    )

    def win(r, k):
        if mw[r][k] is None:
            return None
        mlo, mhi = mw[r][k]
        return mlo * 128, (mhi - mlo) * 128

    awidth = 128
    for r in range(RPC):
        for k in range(KT):
            if mw[r][k]:
                awidth = max(awidth, (mw[r][k][1] - mw[r][k][0]) * 128)

    any_empty_m = any(
        kl[r][m] is None for r in range(RPC) for m in range(mtiles[r])
    )

    with tile.TileContext(nc) as tc:
        with (
            tc.tile_pool(name="const", bufs=1) as cpool,
            tc.tile_pool(name="E", bufs=ebufs) as epool,
            tc.tile_pool(name="bc", bufs=avbufs) as bcpool,
            tc.tile_pool(name="A", bufs=abufs) as apool,
            tc.tile_pool(name="outsb", bufs=obufs) as opool,
            tc.tile_pool(name="psum", bufs=psbufs, space="PSUM") as pspool,
        ):
            # constant column-index tile J[p, j] = j (fp16 ints exact to 2048)
            j_i = cpool.tile([128, awidth], i32)
            nc.gpsimd.iota(j_i[:], pattern=[[1, awidth]], base=0, channel_multiplier=0)
            j_f = cpool.tile([128, awidth], f16 if jf16 else f32)
            nc.vector.tensor_copy(j_f[:], j_i[:])
            a_eng = {"vector": nc.vector, "gpsimd": nc.gpsimd,
                     "scalar": nc.scalar}[aeng]
            if any_empty_m or stage < 3:
                zeros = cpool.tile([128, D], f16)
                nc.vector.memset(zeros[:], 0.0)
            econst = None
            if nodma:
                econst = []
                for h in range(2):
                    tt = cpool.tile([128, 4 * D], f16, tag=f"Ec{h}")
                    nc.vector.memset(tt[:], 0.5)
                    econst.append(tt)

            last_at = None
            ndrain = 0
            for _ in range(repeat):
                if stage >= 0:
                    av = bcpool.tile([128, RPC * KT * 2], f32, tag="av")
                    nc.sync.dma_start(av[:], av_in[:, :])

                for r in range(RPC):
                    scr = scs[r]
                    ktr = (scr + 127) // 128  # k-tiles this slot (last may be partial)
                    rows_of = lambda k: min(128, scr - k * 128)
                    # E row: batched DMAs of up to 4 full k-tiles, partial tail alone
                    et = []
                    nfull = scr // 128
                    rlast = scr - nfull * 128
                    if nodma:
                        for kk in range(ktr):
                            et.append(econst[(kk % 8) // 4][:, (kk % 4) * D : (kk % 4 + 1) * D])
                    elif packed:
                        # contiguous multi-KB lines per partition; split the
                        # full-chunk prefix in two for pipelining, partial
                        # last k-tile rows loaded separately (exact bytes)
                        t = epool.tile([128, KT * D], f16, tag="E")
                        c0 = min(4, nfull)
                        if c0:
                            nc.sync.dma_start(
                                t[:, : c0 * D], E_in[r, :, : c0 * D]
                            )
                        if nfull > c0:
                            nc.sync.dma_start(
                                t[:, c0 * D : nfull * D],
                                E_in[r, :, c0 * D : nfull * D],
                            )
                        if rlast:
                            nc.sync.dma_start(
                                t[:rlast, nfull * D : ktr * D],
                                E_in[r, :rlast, nfull * D : ktr * D],
                            )
                        for k in range(ktr):
                            et.append(t[:, k * D : (k + 1) * D])
                    else:
                        k = 0
                        while k < ktr:
                            kk = min(8 if one_e else 4, nfull - k)
                            if kk > 0:
                                t = epool.tile([128, kk * D], f16, tag="E")
                                src = E_in[
                                    r, k * 128 : (k + kk) * 128, :
                                ].rearrange("(k p) d -> p k d", p=128)
                                nc.sync.dma_start(
                                    t[:].rearrange("p (k d) -> p k d", d=D), src
                                )
                                for i in range(kk):
                                    et.append(t[:, i * D : (i + 1) * D])
                                k += kk
                            else:
                                rows = scr - k * 128
                                t = epool.tile([128, D], f16, tag="Ep")
                                nc.sync.dma_start(
                                    t[:rows], E_in[r, k * 128 : scr, :]
                                )
                                et.append(t[:, :])
                                k += 1

                    # one-hot A windows, one fused DVE op per k-tile
                    ak = {}
                    for k in range(ktr if stage >= 1 else 0):
                        w = win(r, k)
                        if w is None:
                            continue
                        j0, wd = w
                        rows = rows_of(k)
                        c = (r * KT + k) * 2
                        at = apool.tile([128, awidth], f16, tag="A")
                        a_eng.tensor_scalar(
                            at[:rows, :wd],
                            j_f[:rows, :wd],
                            av[:rows, c : c + 1],
                            av[:rows, c + 1 : c + 2],
                            OP.is_equal,
                            OP.mult,
                        )
                        ak[k] = (at, j0)
                        last_at = at

                    mt = mtiles[r]
                    wosb = None
                    if (packed or wide_out) and stage >= 3:
                        wosb = opool.tile([128, MTC * D], f16, tag="wosb")
                    for m in range(mt):
                        if kl[r][m] is None or stage < 2:
                            if outdma and wosb is None:
                                if packed:
                                    wdma(out[r, :, m * D : (m + 1) * D], zeros[:])
                                else:
                                    wdma(
                                        out[r, m * 128 : (m + 1) * 128, :],
                                        zeros[:],
                                    )
                            elif wosb is not None:
                                nc.vector.memset(wosb[:, m * D : (m + 1) * D], 0.0)
                            continue
                        klo, khi = kl[r][m]
                        ps = pspool.tile([128, D], f32, tag="ps")
                        for k in range(klo, khi):
                            at, j0 = ak[k]
                            rows = rows_of(k)
                            lhsT = at[:rows, m * 128 - j0 : (m + 1) * 128 - j0]
                            first = k == klo
                            last = k == khi - 1
                            for n0 in range(0, D, 512):
                                n1 = min(n0 + 512, D)
                                nc.tensor.matmul(
                                    ps[:, n0:n1],
                                    lhsT,
                                    et[k][:rows, n0:n1],
                                    start=first,
                                    stop=last,
                                )
                        if stage < 3:
                            if outdma:
                                if packed:
                                    wdma(out[r, :, m * D : (m + 1) * D], zeros[:])
                                else:
                                    wdma(
                                        out[r, m * 128 : (m + 1) * 128, :],
                                        zeros[:],
                                    )
                            continue
                        if wosb is not None:
                            osb = wosb[:, m * D : (m + 1) * D]
                        else:
                            osbt = opool.tile([128, D], f16, tag="osb")
                            osb = osbt[:]
                        use_act = drain == "act" or (drain == "alt" and ndrain % 2 == 0)
                        ndrain += 1
                        if use_act:
                            nc.scalar.activation(osb, ps[:], AF.Copy)
                        else:
                            nc.vector.tensor_copy(osb, ps[:])
                        if outdma and wosb is None:
                            hi = min((m + 1) * 128, maxnv[r])
                            rows = hi - m * 128
                            if rows > 0:
                                wdma(
                                    out[r, m * 128 : hi, :],
                                    osb[:rows] if rows < 128 else osb,
                                )
                    if outdma and wosb is not None:
                        full_mt = maxnv[r] // 128
                        rows = maxnv[r] - full_mt * 128
                        if packed:
                            if full_mt:
                                wdma(
                                    out[r, :, : full_mt * D],
                                    wosb[:, : full_mt * D],
                                )
                            if rows:
                                wdma(
                                    out[r, :rows, full_mt * D : (full_mt + 1) * D],
                                    wosb[:rows, full_mt * D : (full_mt + 1) * D],
                                )
                        else:
                            if full_mt:
                                wdma(
                                    out[r, : full_mt * 128, :].rearrange(
                                        "(m p) d -> p m d", p=128
                                    ),
                                    wosb[:, : full_mt * D].rearrange(
                                        "p (m d) -> p m d", d=D
                                    ),
                                )
                            if rows:
                                wdma(
                                    out[r, full_mt * 128 : maxnv[r], :],
                                    wosb[:rows, full_mt * D : (full_mt + 1) * D],
                                )

            if tok is not None:
                if last_at is not None:
                    nc.sync.dma_start(tok[:, :8], last_at[:, :16].bitcast(f32))
                else:
                    nc.sync.dma_start(tok[:, :8], zeros[:, :16].bitcast(f32))

    nc.compile()
    return nc


def _prep_full(bert_embedding, x_bert_offset, x_mask, packed=True):
    st = x_bert_offset[..., 0].astype(np.int64)
    ed = x_bert_offset[..., 1].astype(np.int64)
    length = ed - st
    valid = (x_mask > 0) & (length > 0)  # [B, W]
    scale = np.where(
        valid, 1.0 / np.maximum(length, 1).astype(np.float64), 0.0
    ).astype(np.float32)

    # compact word axis: keep only valid words
    cidx = np.where(valid, np.cumsum(valid, axis=1) - 1, -1)  # [B, W]
    nv = valid.sum(axis=1).astype(np.int64)  # [B]

    # word index of each position (-1 if uncovered), then compacted
    st_ext = np.concatenate([st, ed[:, -1:]], axis=1)  # [B, W+1]
    word_of = np.full((B, S), -1, dtype=np.int64)
    s_idx = np.arange(S)
    for b in range(B):
        j = np.searchsorted(st_ext[b], s_idx, side="right") - 1
        ok = (j >= 0) & (j < W)
        word_of[b] = np.where(ok, j, -1)
    wsafe = np.clip(word_of, 0, W - 1)
    covered = word_of >= 0
    bidx = np.arange(B)[:, None]
    cword_of = np.where(covered & valid[bidx, wsafe], cidx[bidx, wsafe], -1)  # [B, S]
    cscale = np.where(cword_of >= 0, scale[bidx, wsafe], 0.0).astype(np.float32)

    # position compaction: the device only reads positions covered by a
    # valid word (~90%); ship E with those rows gathered to the front.
    # ccw/ccs are the per-compacted-rank word index / scale; ranks beyond
    # ncov[b] are padding (idx -1, val 0, E rows unread garbage).
    cov = cword_of >= 0  # [B, S]
    ncov = cov.sum(axis=1).astype(np.int64)  # [B]
    pos_list = [np.nonzero(cov[b])[0] for b in range(B)]
    ccw = np.full((B, S), -1, dtype=np.int64)
    ccs = np.zeros((B, S), dtype=np.float32)
    for b in range(B):
        n = int(ncov[b])
        ccw[b, :n] = cword_of[b, pos_list[b]]
        ccs[b, :n] = cscale[b, pos_list[b]]
    scs = tuple(
        max(int(ncov[c * RPC + r]) for c in range(NCORES)) for r in range(RPC)
    )

    # per row-slot r: number of m-tiles = max over cores of ceil(nv/128)
    mtiles = []
    for r in range(RPC):
        mt = 1
        for c in range(NCORES):
            mt = max(mt, int(-(-nv[c * RPC + r] // 128)))
        mtiles.append(mt)

    # kl[r][m]: hull of active k-tiles (over compacted position ranks) per
    # compacted m-tile, unioned over cores
    kl = []
    for r in range(RPC):
        per_m = []
        for m in range(mtiles[r]):
            klo, khi = KT, 0
            for c in range(NCORES):
                b = c * RPC + r
                sel = (ccw[b] >= m * 128) & (ccw[b] < (m + 1) * 128)
                if sel.any():
                    ss = np.nonzero(sel)[0]
                    klo = min(klo, int(ss[0]) // 128)
                    khi = max(khi, int(ss[-1]) // 128 + 1)
            per_m.append((klo, khi) if khi > klo else None)
        kl.append(per_m)

    # mw[r][k]: hull of m-tiles whose kl-range contains k (guarantees every
    # matmul slice lies inside the built A window)
    mw = []
    for r in range(RPC):
        per_k = []
        for k in range(KT):
            mlo, mhi = mtiles[r], 0
            for m in range(mtiles[r]):
                if kl[r][m] and kl[r][m][0] <= k < kl[r][m][1]:
                    mlo = min(mlo, m)
                    mhi = max(mhi, m + 1)
            per_k.append((mlo, mhi) if mhi > mlo else None)
        mw.append(per_k)

    maxnv = tuple(
        max(int(nv[c * RPC + r]) for c in range(NCORES)) for r in range(RPC)
    )
    pairs = (kl, mw, tuple(mtiles), maxnv, scs)

    E16 = bert_embedding.astype(np.float16)
    in_maps = []
    for c in range(NCORES):
        av = np.zeros((128, RPC * KT * 2), dtype=np.float32)
        if packed:
            E_dev = np.zeros((RPC, 128, KT * D), dtype=np.float16)
        else:
            E_dev = np.zeros((RPC, S, D), dtype=np.float16)
        for r in range(RPC):
            b = c * RPC + r
            n = int(ncov[b])
            gath = E16[b, pos_list[b]]  # [n, D]
            if packed:
                # position k*128+p -> [p, k*D:(k+1)*D]
                for k in range((n + 127) // 128):
                    rows = min(128, n - k * 128)
                    E_dev[r, :rows, k * D : (k + 1) * D] = gath[
                        k * 128 : k * 128 + rows
                    ]
            else:
                E_dev[r, :n] = gath
            ktr = (scs[r] + 127) // 128
            for k in range(ktr):
                if mw[r][k] is None:
                    continue
                j0 = mw[r][k][0] * 128
                col = (r * KT + k) * 2
                i = k * 128 + np.arange(128)
                cw = ccw[b, i]
                covk = cw >= 0
                # window hull guarantees covered words lie inside [j0, j0+wd)
                av[:, col] = np.where(covk, cw - j0, -1).astype(np.float32)
                av[:, col + 1] = ccs[b, i].astype(np.float32)
        in_maps.append(
            {
                "E_in": E_dev,
                "av_in": av,
            }
        )
    scatter = (valid, cidx, nv)
    return pairs, in_maps, scatter


def _prep(bert_embedding, x_bert_offset, x_mask, packed=True):
    pairs, in_maps, _ = _prep_full(
        bert_embedding, x_bert_offset, x_mask, packed=packed
    )
    return pairs, in_maps


def kernel(bert_embedding, x_bert_offset, x_mask):
    from concourse.bass_utils import run_bass_kernel_spmd

    bert_embedding = np.asarray(bert_embedding, dtype=np.float32)
    x_bert_offset = np.asarray(x_bert_offset)
    x_mask = np.asarray(x_mask)
    pairs, in_maps, scatter = _prep_full(bert_embedding, x_bert_offset, x_mask)
    key = repr(pairs)
    nc = _CACHE.get(key)
    if nc is None:
        nc = build_program(pairs)
        _CACHE[key] = nc
    res = run_bass_kernel_spmd(nc, in_maps, list(range(NCORES)))
    MTC = max(pairs[2])
    comp = np.concatenate(
        [
            np.asarray(res.results[c]["out"])
            .reshape(RPC, 128, MTC, D)
            .transpose(0, 2, 1, 3)
            .reshape(RPC, MTC * 128, D)
            for c in range(NCORES)
        ],
        axis=0,
    )  # [B, MTC*128, D] fp16, compacted word rows

    valid, cidx, nv = scatter
    gather_idx = np.clip(cidx, 0, comp.shape[1] - 1)[..., None]  # [B, W, 1]
    full = np.take_along_axis(
        comp.astype(np.float32), np.broadcast_to(gather_idx, (B, W, D)), axis=1
    )
    full = np.where(valid[..., None], full, np.float32(0.0))
    return np.ascontiguousarray(full, dtype=np.float32)


# revision 43
# speedup vs baseline: 1.0197x; 1.0197x over previous
"""Trainium2 Bass kernel for ragged subword mean pooling (nn_Bert).

Problem: out[b, j] = mean(bert_embedding[b, st_j:ed_j]) if (mask & ed>st) else 0
Shapes: bert_embedding [32, 1024, 768] f32, x_bert_offset [32, 768, 2] i32,
        x_mask [32, 768] i32 -> out [32, 768, 768] f32.

Strategy (pure data parallel, 4 batch rows per core on 8 cores):
  Spans are contiguous sorted segments, so per row the pooling is
  out = A.T @ E where A[s, j] = scale_j iff st_j <= s < ed_j
  (scale_j = valid/len folds the mean and mask directly into A).
  Each position s belongs to at most ONE word, so every A tile has at
  most one nonzero per partition row. The host ships just that
  (column, value) pair per position and the device reconstructs each
  [128, win] A window in a single fused DVE op against a constant
  column-index tile J:
      A[p, j] = (J[p, j] == idx_p) * val_p

  The kernel is memory-bound, so bytes are minimized three ways:
  1. 16-bit I/O: E is shipped as fp16 (host casts; ~5e-4 rel rounding)
     and the output is written back as fp16 and upcast on the host.
  2. Output compaction: ~37% of words are invalid (masked or empty
     span) and produce zero rows. The word axis is compacted to valid
     words only before building A; the device writes only the rows up
     to the per-slot max valid count (partial last tile) and the host
     scatters rows back to the full [W] axis (invalid rows are zeros).
  3. Position compaction: ~10% of subword positions belong to masked/
     empty words and never contribute. The host gathers only covered-
     by-valid positions to the front of each E row; the device reads
     the per-slot max covered count of lines (partial last k-tile).

  DMA efficiency: the host ships E pre-packed in SBUF layout
  (partition-major, position k*128+p at [r, p, k*D:(k+1)*D]) and the
  output is written partition-major and un-transposed on the host, so
  every DMA moves multi-KB contiguous lines per partition instead of
  1.5 KB strided ones (measured ~20% faster than the strided layout).

  The contraction runs on the PE in fp16 (full rate, f32 PSUM
  accumulate). PSUM is drained by alternating scalar/vector copies.
  Only (m, k) tile pairs whose word/position ranges intersect are
  computed; the active-pair hull is derived on the host from the
  actual offsets (a superset is always correct since A is 0 outside).
"""

import sys

if "/opt/trn_rl_repo" not in sys.path:
    sys.path.insert(0, "/opt/trn_rl_repo")

import numpy as np

B, S, W, D = 32, 1024, 768, 768
NCORES = 8
RPC = B // NCORES  # rows per core
KT = S // 128  # 8 k-tiles (positions)

_CACHE = {}


def build_program(pairs, repeat=1, drain="alt", io="ext", stage=3, nodma=False,
                  ebufs=4, abufs=8, psbufs=3, obufs=3, avbufs=2,
                  wide_out=False, one_e=False, wq=False, packed=True,
                  jf16=False, aeng="vector", nsplit=512):
    """Build the SPMD Bass program (one program, run on all 8 cores)."""
    import concourse.tile as tile
    from concourse import bacc, mybir

    kl, mw, mtiles = pairs[:3]
    maxnv = pairs[3] if len(pairs) > 3 else tuple(m * 128 for m in mtiles)
    # per-slot count of (position-compacted) E lines to read; <= S
    scs = pairs[4] if len(pairs) > 4 else (S,) * RPC
    MTC = max(mtiles)
    f16 = mybir.dt.float16
    f32 = mybir.dt.float32
    i32 = mybir.dt.int32
    AF = mybir.ActivationFunctionType
    OP = mybir.AluOpType

    nc = bacc.Bacc(
        "TRN2", target_bir_lowering=False, debug=False, num_devices=NCORES
    )

    if packed:
        # host pre-packs E into SBUF layout: partition-major, position
        # k*128+p at [r, p, k*D:(k+1)*D]; DMA lines are multi-KB contiguous
        E_in = nc.dram_tensor(
            "E_in", [RPC, 128, KT * D], f16, kind="ExternalInput"
        ).ap()
    else:
        E_in = nc.dram_tensor("E_in", [RPC, S, D], f16, kind="ExternalInput").ap()
    # packed per (r, k): column 2*(r*KT+k) = one-hot column index within the
    # A window (or -1), column +1 = A value (scale of the word at that
    # position, 0 if masked/empty/uncovered)
    av_in = nc.dram_tensor("av_in", [128, RPC * KT * 2], f32, kind="ExternalInput").ap()
    oshape = [RPC, 128, MTC * D] if packed else [RPC, MTC * 128, D]
    if io == "ext":
        out = nc.dram_tensor("out", oshape, f16, kind="ExternalOutput").ap()
        tok = None
    else:
        out = nc.dram_tensor("out_scratch", oshape, f16).ap()
        tok = nc.dram_tensor("tok", [128, 16], f32, kind="ExternalOutput").ap()
    outdma = not nodma
    wdma = (lambda o, i: nc.scalar.dma_start(o, i)) if wq else (
        lambda o, i: nc.sync.dma_start(o, i)
    )

    def win(r, k):
        if mw[r][k] is None:
            return None
        mlo, mhi = mw[r][k]
        return mlo * 128, (mhi - mlo) * 128

    awidth = 128
    for r in range(RPC):
        for k in range(KT):
            if mw[r][k]:
                awidth = max(awidth, (mw[r][k][1] - mw[r][k][0]) * 128)

    any_empty_m = any(
        kl[r][m] is None for r in range(RPC) for m in range(mtiles[r])
    )

    with tile.TileContext(nc) as tc:
        with (
            tc.tile_pool(name="const", bufs=1) as cpool,
            tc.tile_pool(name="E", bufs=ebufs) as epool,
            tc.tile_pool(name="bc", bufs=avbufs) as bcpool,
            tc.tile_pool(name="A", bufs=abufs) as apool,
            tc.tile_pool(name="outsb", bufs=obufs) as opool,
            tc.tile_pool(name="psum", bufs=psbufs, space="PSUM") as pspool,
        ):
            # constant column-index tile J[p, j] = j (fp16 ints exact to 2048)
            j_i = cpool.tile([128, awidth], i32)
            nc.gpsimd.iota(j_i[:], pattern=[[1, awidth]], base=0, channel_multiplier=0)
            j_f = cpool.tile([128, awidth], f16 if jf16 else f32)
            nc.vector.tensor_copy(j_f[:], j_i[:])
            a_eng = {"vector": nc.vector, "gpsimd": nc.gpsimd,
                     "scalar": nc.scalar}[aeng]
            if any_empty_m or stage < 3:
                zeros = cpool.tile([128, D], f16)
                nc.vector.memset(zeros[:], 0.0)
            econst = None
            if nodma:
                econst = []
                for h in range(2):
                    tt = cpool.tile([128, 4 * D], f16, tag=f"Ec{h}")
                    nc.vector.memset(tt[:], 0.5)
                    econst.append(tt)

            last_at = None
            ndrain = 0
            for _ in range(repeat):
                if stage >= 0:
                    av = bcpool.tile([128, RPC * KT * 2], f32, tag="av")
                    nc.sync.dma_start(av[:], av_in[:, :])

                for r in range(RPC):
                    scr = scs[r]
                    ktr = (scr + 127) // 128  # k-tiles this slot (last may be partial)
                    rows_of = lambda k: min(128, scr - k * 128)
                    # E row: batched DMAs of up to 4 full k-tiles, partial tail alone
                    et = []
                    nfull = scr // 128
                    rlast = scr - nfull * 128
                    if nodma:
                        for kk in range(ktr):
                            et.append(econst[(kk % 8) // 4][:, (kk % 4) * D : (kk % 4 + 1) * D])
                    elif packed:
                        # contiguous multi-KB lines per partition; split the
                        # full-chunk prefix in two for pipelining, partial
                        # last k-tile rows loaded separately (exact bytes)
                        t = epool.tile([128, KT * D], f16, tag="E")
                        c0 = min(4, nfull)
                        if c0:
                            nc.sync.dma_start(
                                t[:, : c0 * D], E_in[r, :, : c0 * D]
                            )
                        if nfull > c0:
                            nc.sync.dma_start(
                                t[:, c0 * D : nfull * D],
                                E_in[r, :, c0 * D : nfull * D],
                            )
                        if rlast:
                            nc.sync.dma_start(
                                t[:rlast, nfull * D : ktr * D],
                                E_in[r, :rlast, nfull * D : ktr * D],
                            )
                        for k in range(ktr):
                            et.append(t[:, k * D : (k + 1) * D])
                    else:
                        k = 0
                        while k < ktr:
                            kk = min(8 if one_e else 4, nfull - k)
                            if kk > 0:
                                t = epool.tile([128, kk * D], f16, tag="E")
                                src = E_in[
                                    r, k * 128 : (k + kk) * 128, :
                                ].rearrange("(k p) d -> p k d", p=128)
                                nc.sync.dma_start(
                                    t[:].rearrange("p (k d) -> p k d", d=D), src
                                )
                                for i in range(kk):
                                    et.append(t[:, i * D : (i + 1) * D])
                                k += kk
                            else:
                                rows = scr - k * 128
                                t = epool.tile([128, D], f16, tag="Ep")
                                nc.sync.dma_start(
                                    t[:rows], E_in[r, k * 128 : scr, :]
                                )
                                et.append(t[:, :])
                                k += 1

                    # one-hot A windows, one fused DVE op per k-tile
                    ak = {}
                    for k in range(ktr if stage >= 1 else 0):
                        w = win(r, k)
                        if w is None:
                            continue
                        j0, wd = w
                        rows = rows_of(k)
                        c = (r * KT + k) * 2
                        at = apool.tile([128, awidth], f16, tag="A")
                        a_eng.tensor_scalar(
                            at[:rows, :wd],
                            j_f[:rows, :wd],
                            av[:rows, c : c + 1],
                            av[:rows, c + 1 : c + 2],
                            OP.is_equal,
                            OP.mult,
                        )
                        ak[k] = (at, j0)
                        last_at = at

                    mt = mtiles[r]
                    wosb = None
                    if (packed or wide_out) and stage >= 3:
                        wosb = opool.tile([128, MTC * D], f16, tag="wosb")
                    for m in range(mt):
                        if kl[r][m] is None or stage < 2:
                            if outdma and wosb is None:
                                if packed:
                                    wdma(out[r, :, m * D : (m + 1) * D], zeros[:])
                                else:
                                    wdma(
                                        out[r, m * 128 : (m + 1) * 128, :],
                                        zeros[:],
                                    )
                            elif wosb is not None:
                                nc.vector.memset(wosb[:, m * D : (m + 1) * D], 0.0)
                            continue
                        klo, khi = kl[r][m]
                        ps = pspool.tile([128, D], f32, tag="ps")
                        for k in range(klo, khi):
                            at, j0 = ak[k]
                            rows = rows_of(k)
                            lhsT = at[:rows, m * 128 - j0 : (m + 1) * 128 - j0]
                            first = k == klo
                            last = k == khi - 1
                            for n0 in range(0, D, nsplit):
                                n1 = min(n0 + nsplit, D)
                                nc.tensor.matmul(
                                    ps[:, n0:n1],
                                    lhsT,
                                    et[k][:rows, n0:n1],
                                    start=first,
                                    stop=last,
                                )
                        if stage < 3:
                            if outdma:
                                if packed:
                                    wdma(out[r, :, m * D : (m + 1) * D], zeros[:])
                                else:
                                    wdma(
                                        out[r, m * 128 : (m + 1) * 128, :],
                                        zeros[:],
                                    )
                            continue
                        if wosb is not None:
                            osb = wosb[:, m * D : (m + 1) * D]
                        else:
                            osbt = opool.tile([128, D], f16, tag="osb")
                            osb = osbt[:]
                        use_act = drain == "act" or (drain == "alt" and ndrain % 2 == 0)
                        ndrain += 1
                        if use_act:
                            nc.scalar.activation(osb, ps[:], AF.Copy)
                        else:
                            nc.vector.tensor_copy(osb, ps[:])
                        if outdma and wosb is None:
                            hi = min((m + 1) * 128, maxnv[r])
                            rows = hi - m * 128
                            if rows > 0:
                                wdma(
                                    out[r, m * 128 : hi, :],
                                    osb[:rows] if rows < 128 else osb,
                                )
                    if outdma and wosb is not None:
                        full_mt = maxnv[r] // 128
                        rows = maxnv[r] - full_mt * 128
                        if packed:
                            if full_mt:
                                wdma(
                                    out[r, :, : full_mt * D],
                                    wosb[:, : full_mt * D],
                                )
                            if rows:
                                wdma(
                                    out[r, :rows, full_mt * D : (full_mt + 1) * D],
                                    wosb[:rows, full_mt * D : (full_mt + 1) * D],
                                )
                        else:
                            if full_mt:
                                wdma(
                                    out[r, : full_mt * 128, :].rearrange(
                                        "(m p) d -> p m d", p=128
                                    ),
                                    wosb[:, : full_mt * D].rearrange(
                                        "p (m d) -> p m d", d=D
                                    ),
                                )
                            if rows:
                                wdma(
                                    out[r, full_mt * 128 : maxnv[r], :],
                                    wosb[:rows, full_mt * D : (full_mt + 1) * D],
                                )

            if tok is not None:
                if last_at is not None:
                    nc.sync.dma_start(tok[:, :8], last_at[:, :16].bitcast(f32))
                else:
                    nc.sync.dma_start(tok[:, :8], zeros[:, :16].bitcast(f32))

    nc.compile()
    return nc


def _prep_full(bert_embedding, x_bert_offset, x_mask, packed=True):
    st = x_bert_offset[..., 0].astype(np.int64)
    ed = x_bert_offset[..., 1].astype(np.int64)
    length = ed - st
    valid = (x_mask > 0) & (length > 0)  # [B, W]
    scale = np.where(
        valid, 1.0 / np.maximum(length, 1).astype(np.float64), 0.0
    ).astype(np.float32)

    # compact word axis: keep only valid words
    cidx = np.where(valid, np.cumsum(valid, axis=1) - 1, -1)  # [B, W]
    nv = valid.sum(axis=1).astype(np.int64)  # [B]

    # word index of each position (-1 if uncovered), then compacted
    st_ext = np.concatenate([st, ed[:, -1:]], axis=1)  # [B, W+1]
    word_of = np.full((B, S), -1, dtype=np.int64)
    s_idx = np.arange(S)
    for b in range(B):
        j = np.searchsorted(st_ext[b], s_idx, side="right") - 1
        ok = (j >= 0) & (j < W)
        word_of[b] = np.where(ok, j, -1)
    wsafe = np.clip(word_of, 0, W - 1)
    covered = word_of >= 0
    bidx = np.arange(B)[:, None]
    cword_of = np.where(covered & valid[bidx, wsafe], cidx[bidx, wsafe], -1)  # [B, S]
    cscale = np.where(cword_of >= 0, scale[bidx, wsafe], 0.0).astype(np.float32)

    # position compaction: the device only reads positions covered by a
    # valid word (~90%); ship E with those rows gathered to the front.
    # ccw/ccs are the per-compacted-rank word index / scale; ranks beyond
    # ncov[b] are padding (idx -1, val 0, E rows unread garbage).
    cov = cword_of >= 0  # [B, S]
    ncov = cov.sum(axis=1).astype(np.int64)  # [B]
    pos_list = [np.nonzero(cov[b])[0] for b in range(B)]
    ccw = np.full((B, S), -1, dtype=np.int64)
    ccs = np.zeros((B, S), dtype=np.float32)
    for b in range(B):
        n = int(ncov[b])
        ccw[b, :n] = cword_of[b, pos_list[b]]
        ccs[b, :n] = cscale[b, pos_list[b]]
    scs = tuple(
        max(int(ncov[c * RPC + r]) for c in range(NCORES)) for r in range(RPC)
    )

    # per row-slot r: number of m-tiles = max over cores of ceil(nv/128)
    mtiles = []
    for r in range(RPC):
        mt = 1
        for c in range(NCORES):
            mt = max(mt, int(-(-nv[c * RPC + r] // 128)))
        mtiles.append(mt)

    # kl[r][m]: hull of active k-tiles (over compacted position ranks) per
    # compacted m-tile, unioned over cores
    kl = []
    for r in range(RPC):
        per_m = []
        for m in range(mtiles[r]):
            klo, khi = KT, 0
            for c in range(NCORES):
                b = c * RPC + r
                sel = (ccw[b] >= m * 128) & (ccw[b] < (m + 1) * 128)
                if sel.any():
                    ss = np.nonzero(sel)[0]
                    klo = min(klo, int(ss[0]) // 128)
                    khi = max(khi, int(ss[-1]) // 128 + 1)
            per_m.append((klo, khi) if khi > klo else None)
        kl.append(per_m)

    # mw[r][k]: hull of m-tiles whose kl-range contains k (guarantees every
    # matmul slice lies inside the built A window)
    mw = []
    for r in range(RPC):
        per_k = []
        for k in range(KT):
            mlo, mhi = mtiles[r], 0
            for m in range(mtiles[r]):
                if kl[r][m] and kl[r][m][0] <= k < kl[r][m][1]:
                    mlo = min(mlo, m)
                    mhi = max(mhi, m + 1)
            per_k.append((mlo, mhi) if mhi > mlo else None)
        mw.append(per_k)

    maxnv = tuple(
        max(int(nv[c * RPC + r]) for c in range(NCORES)) for r in range(RPC)
    )
    pairs = (kl, mw, tuple(mtiles), maxnv, scs)

    E16 = bert_embedding.astype(np.float16)
    in_maps = []
    for c in range(NCORES):
        av = np.zeros((128, RPC * KT * 2), dtype=np.float32)
        if packed:
            E_dev = np.zeros((RPC, 128, KT * D), dtype=np.float16)
        else:
            E_dev = np.zeros((RPC, S, D), dtype=np.float16)
        for r in range(RPC):
            b = c * RPC + r
            n = int(ncov[b])
            gath = E16[b, pos_list[b]]  # [n, D]
            if packed:
                # position k*128+p -> [p, k*D:(k+1)*D]
                for k in range((n + 127) // 128):
                    rows = min(128, n - k * 128)
                    E_dev[r, :rows, k * D : (k + 1) * D] = gath[
                        k * 128 : k * 128 + rows
                    ]
            else:
                E_dev[r, :n] = gath
            ktr = (scs[r] + 127) // 128
            for k in range(ktr):
                if mw[r][k] is None:
                    continue
                j0 = mw[r][k][0] * 128
                col = (r * KT + k) * 2
                i = k * 128 + np.arange(128)
                cw = ccw[b, i]
                covk = cw >= 0
                # window hull guarantees covered words lie inside [j0, j0+wd)
                av[:, col] = np.where(covk, cw - j0, -1).astype(np.float32)
                av[:, col + 1] = ccs[b, i].astype(np.float32)
        in_maps.append(
            {
                "E_in": E_dev,
                "av_in": av,
            }
        )
    scatter = (valid, cidx, nv)
    return pairs, in_maps, scatter


def _prep(bert_embedding, x_bert_offset, x_mask, packed=True):
    pairs, in_maps, _ = _prep_full(
        bert_embedding, x_bert_offset, x_mask, packed=packed
    )
    return pairs, in_maps


def kernel(bert_embedding, x_bert_offset, x_mask):
    from concourse.bass_utils import run_bass_kernel_spmd

    bert_embedding = np.asarray(bert_embedding, dtype=np.float32)
    x_bert_offset = np.asarray(x_bert_offset)
    x_mask = np.asarray(x_mask)
    pairs, in_maps, scatter = _prep_full(bert_embedding, x_bert_offset, x_mask)
    key = repr(pairs)
    nc = _CACHE.get(key)
    if nc is None:
        nc = build_program(pairs)
        _CACHE[key] = nc
    res = run_bass_kernel_spmd(nc, in_maps, list(range(NCORES)))
    MTC = max(pairs[2])
    comp = np.concatenate(
        [
            np.asarray(res.results[c]["out"])
            .reshape(RPC, 128, MTC, D)
            .transpose(0, 2, 1, 3)
            .reshape(RPC, MTC * 128, D)
            for c in range(NCORES)
        ],
        axis=0,
    )  # [B, MTC*128, D] fp16, compacted word rows

    valid, cidx, nv = scatter
    gather_idx = np.clip(cidx, 0, comp.shape[1] - 1)[..., None]  # [B, W, 1]
    full = np.take_along_axis(
        comp.astype(np.float32), np.broadcast_to(gather_idx, (B, W, D)), axis=1
    )
    full = np.where(valid[..., None], full, np.float32(0.0))
    return np.ascontiguousarray(full, dtype=np.float32)
